# revision 1
# baseline (speedup 1.0000x reference)
"""Trainium2 Bass kernel for nn_AttentionModule (dense transformer block).

Computation (per batch element b):
    q = X @ Wq.T ; k = K @ Wk.T ; v = X @ Wv.T        (X=query_input, K=key_input)
    a = softmax((k @ q.T) / sqrt(D), axis=-1)          -> (NK, NQ)
    out = a @ v + K                                    -> (NK, D)

Sharding: data-parallel over batch, one batch element per NeuronCore (B == 8).

Layout strategy: matmul contractions run on the partition axis, so the host
pre-transposes X, K and the weights to feature-major layouts (and rounds them
to bf16 — partial sums stay fp32 in PSUM, and the residual add of key_input is
done in fp32, keeping output error at the ~1e-4 level). The kernel computes
qT/kT projections (kT and v spilled to DRAM), scores in [n_q, n_k] layout, exp
on the scalar engine, the softmax denominator with a ones-vector matmul, and
the context matmul consumes exp(S)T directly as the stationary operand. The
normalization is folded into the output pass as a fused per-partition
multiply-add on the vector engine.
"""

import numpy as np
import ml_dtypes

import concourse.tile as tile
from concourse import bacc, mybir
from concourse.bass_utils import run_bass_kernel_spmd
from concourse.masks import make_identity

B, NQ, NK, D = 8, 2048, 2048, 1024
P = 128
EB = D // P          # 8 feature blocks
NB = NQ // P         # 16 query-row blocks
MC = 512             # scores chunk width (n_k columns per chunk)
NMC = NK // MC       # 4 chunks
SCALE = 1.0 / float(np.sqrt(np.float32(D)))

F32 = mybir.dt.float32
BF16 = mybir.dt.bfloat16

_CACHE = {}


def _build():
    nc = bacc.Bacc("TRN2", target_bir_lowering=False, debug=False, num_devices=B)

    xT = nc.dram_tensor("xT", [D, NQ], BF16, kind="ExternalInput").ap()
    ktT = nc.dram_tensor("ktT", [D, NK], BF16, kind="ExternalInput").ap()
    knat = nc.dram_tensor("knat", [NK, D], F32, kind="ExternalInput").ap()
    wqT = nc.dram_tensor("wqT", [D, D], BF16, kind="ExternalInput").ap()
    wkT = nc.dram_tensor("wkT", [D, D], BF16, kind="ExternalInput").ap()
    wvT = nc.dram_tensor("wvT", [D, D], BF16, kind="ExternalInput").ap()
    out = nc.dram_tensor("out", [NK, D], F32, kind="ExternalOutput").ap()

    with tile.TileContext(nc) as tc:
        with (
            tc.tile_pool(name="const", bufs=1) as constp,
            tc.tile_pool(name="qt", bufs=EB) as qtp,
            tc.tile_pool(name="dram", bufs=1, space="DRAM") as dramp,
            tc.tile_pool(name="psum", bufs=1, space="PSUM") as psp,
            tc.tile_pool(name="stage", bufs=12) as stagep,
            tc.tile_pool(name="ktc", bufs=18) as ktcp,
        ):
            ident = constp.tile([1, 1], F32, tag="ident", name="ident")
            make_identity(nc, ident)
            ones = constp.tile([P, 1], BF16, tag="ones", name="ones")
            nc.vector.memset(ones, 1.0)

            kT_sp = dramp.tile([D, NK], BF16, tag="kT_sp", name="kT_sp")
            v_sp = dramp.tile([NQ, D], BF16, tag="v_sp", name="v_sp")

            qT = [qtp.tile([P, NQ], BF16, tag="qt", name="qt") for _ in range(EB)]

            # ---------------- phase 1: projections ----------------
            with (
                tc.tile_pool(name="bigin", bufs=16) as bigp,
                tc.tile_pool(name="wpool", bufs=16) as wp,
            ):
                # -- kT[e, m] = sum_d wkT[d, e] * ktT[d, m]  (spilled to DRAM)
                # ktT/wkT are loaded in column halves so the first matmul
                # group only waits on half the bytes (shorter pipeline fill).
                # loads are emitted in first-consumed order: wk first half,
                # then ktT quarters in consumption order, wk second half last.
                # The first matmul group only waits on ~2MB.
                wk_h = [[None] * 2 for _ in range(EB)]
                kt_q = [[None] * 4 for _ in range(EB)]
                for db in range(EB):
                    t = wp.tile([P, D // 2], BF16, tag="wh", name="wh", bufs=16)
                    nc.sync.dma_start(
                        out=t, in_=wkT[db * P:(db + 1) * P, 0:512]
                    )
                    wk_h[db][0] = t
                for q in range(4):
                    for db in range(EB):
                        t = bigp.tile([P, NK // 4], BF16, tag="kth", name="kth", bufs=32)
                        nc.sync.dma_start(
                            out=t,
                            in_=ktT[db * P:(db + 1) * P, q * 512:(q + 1) * 512],
                        )
                        kt_q[db][q] = t
                for db in range(EB):
                    t = wp.tile([P, D // 2], BF16, tag="wh", name="wh", bufs=16)
                    nc.sync.dma_start(
                        out=t, in_=wkT[db * P:(db + 1) * P, 512:1024]
                    )
                    wk_h[db][1] = t
                gi = 0
                for h2 in range(2):
                    for mc4 in range(NK // 512):
                        for eb in range(h2 * 4, h2 * 4 + 4):
                            tg = "mm" if gi % 2 == 0 else "st"
                            gi += 1
                            ps = psp.tile([P, 512], F32, tag=tg, name="mm",
                                          bufs=3 if tg == "mm" else 4)
                            for db in range(EB):
                                nc.tensor.matmul(
                                    ps,
                                    wk_h[db][h2][:, (eb % 4) * P:(eb % 4 + 1) * P],
                                    kt_q[db][mc4],
                                    start=(db == 0),
                                    stop=(db == EB - 1),
                                )
                            st = stagep.tile([P, 512], BF16, tag="stage", name="stage")
                            nc.vector.tensor_copy(st, ps)
                            nc.scalar.dma_start(
                                out=kT_sp[eb * P:(eb + 1) * P, mc4 * 512:(mc4 + 1) * 512],
                                in_=st,
                            )

                # prefetch chunk-0 score operands while qT/v phases run
                ktc0 = []
                for eb in range(EB):
                    t = ktcp.tile([P, MC], BF16, tag="ktc", name="ktc")
                    nc.sync.dma_start(out=t, in_=kT_sp[eb * P:(eb + 1) * P, 0:MC])
                    ktc0.append(t)

                # -- qT[e, n] = sum_d wqT[d, e] * xT[d, n]  (SBUF resident)
                x_in = []
                for db in range(EB):
                    t = bigp.tile([P, NQ], BF16, tag="big", name="big", bufs=8)
                    nc.sync.dma_start(out=t, in_=xT[db * P:(db + 1) * P, :])
                    x_in.append(t)
                wq = []
                for db in range(EB):
                    t = wp.tile([P, D], BF16, tag="w", name="w", bufs=16)
                    nc.sync.dma_start(out=t, in_=wqT[db * P:(db + 1) * P, :])
                    wq.append(t)
                for eb in range(EB):
                    for nc4 in range(NQ // 512):
                        tg = "mm" if (eb * 4 + nc4) % 2 == 0 else "st"
                        ps = psp.tile([P, 512], F32, tag=tg, name="mm",
                                      bufs=3 if tg == "mm" else 4)
                        for db in range(EB):
                            nc.tensor.matmul(
                                ps,
                                wq[db][:, eb * P:(eb + 1) * P],
                                x_in[db][:, nc4 * 512:(nc4 + 1) * 512],
                                start=(db == 0),
                                stop=(db == EB - 1),
                            )
                        nc.vector.tensor_copy(
                            qT[eb][:, nc4 * 512:(nc4 + 1) * 512], ps
                        )

                # -- v[n, dv] = sum_d xT[d, n] * wvT[d, dv]  (spilled to DRAM)
                wv = []
                for db in range(EB):
                    t = wp.tile([P, D], BF16, tag="w", name="w", bufs=16)
                    nc.sync.dma_start(out=t, in_=wvT[db * P:(db + 1) * P, :])
                    wv.append(t)
                for nb in range(NB):
                    for dc in range(D // 512):
                        tg = "mm" if (nb * 2 + dc) % 2 == 0 else "st"
                        ps = psp.tile([P, 512], F32, tag=tg, name="mm",
                                      bufs=3 if tg == "mm" else 4)
                        for db in range(EB):
                            nc.tensor.matmul(
                                ps,
                                x_in[db][:, nb * P:(nb + 1) * P],
                                wv[db][:, dc * 512:(dc + 1) * 512],
                                start=(db == 0),
                                stop=(db == EB - 1),
                            )
                        st = stagep.tile([P, 512], BF16, tag="stage", name="stage")
                        nc.vector.tensor_copy(st, ps)
                        nc.scalar.dma_start(
                            out=v_sp[nb * P:(nb + 1) * P, dc * 512:(dc + 1) * 512],
                            in_=st,
                        )

            # ---------------- phase 2: attention ----------------
            with (
                tc.tile_pool(name="expst", bufs=18) as expp,
                tc.tile_pool(name="vst", bufs=20) as vstp,
                tc.tile_pool(name="knp", bufs=6) as knp,
                tc.tile_pool(name="outp", bufs=6) as outp,
                tc.tile_pool(name="small", bufs=4) as smallp,
            ):
                for mc in range(NMC):
                    m0 = mc * MC
                    if mc == 0:
                        ktc = ktc0
                    else:
                        ktc = []
                        for eb in range(EB):
                            t = ktcp.tile([P, MC], BF16, tag="ktc", name="ktc")
                            nc.sync.dma_start(
                                out=t, in_=kT_sp[eb * P:(eb + 1) * P, m0:m0 + MC]
                            )
                            ktc.append(t)

                    # scores + exp + column-sum accumulation
                    expst = []
                    cs_ps = psp.tile([1, MC], F32, tag="csrp", name="cs", bufs=1)
                    for nb in range(NB):
                        st_ps = psp.tile([P, MC], F32, tag="st", name="st", bufs=4)
                        for eb in range(EB):
                            nc.tensor.matmul(
                                st_ps,
                                qT[eb][:, nb * P:(nb + 1) * P],
                                ktc[eb],
                                start=(eb == 0),
                                stop=(eb == EB - 1),
                            )
                        et = expp.tile([P, MC], BF16, tag="expst", name="expst")
                        nc.scalar.activation(
                            out=et, in_=st_ps,
                            func=mybir.ActivationFunctionType.Exp, scale=SCALE,
                        )
                        expst.append(et)
                        # the column-sum matmul for block j is emitted two
                        # groups late so the exp -> cs semaphore never gates PE
                        if nb >= 2:
                            j = nb - 2
                            nc.tensor.matmul(
                                cs_ps, ones, expst[j],
                                start=(j == 0), stop=False,
                            )

                    for j in (NB - 2, NB - 1):
                        nc.tensor.matmul(
                            cs_ps, ones, expst[j],
                            start=False, stop=(j == NB - 1),
                        )
                    recip_row = smallp.tile([1, MC], F32, tag="rrow", name="rrow")
                    nc.vector.reciprocal(recip_row, cs_ps)
                    rp_ps = psp.tile([P, MC // P], F32, tag="csrp", name="rp", bufs=1)
                    for j in range(MC // P):
                        nc.tensor.transpose(
                            rp_ps[:, j:j + 1],
                            recip_row[:, j * P:(j + 1) * P],
                            ident,
                        )
                    recip_pp = smallp.tile([P, MC // P], F32, tag="rpp", name="rpp")
                    nc.vector.tensor_copy(recip_pp, rp_ps)

                    # context: C[m, dv] = sum_n expst[n, m] * v[n, dv]
                    vts = []
                    for nb in range(NB):
                        vt = vstp.tile([P, D], BF16, tag="vst", name="vst")
                        nc.sync.dma_start(
                            out=vt, in_=v_sp[nb * P:(nb + 1) * P, :],
                        )
                        vts.append(vt)
                    for msb in range(MC // P):
                        r0 = m0 + msb * P
                        kn = knp.tile([P, D], F32, tag="knat", name="knat")
                        nc.sync.dma_start(out=kn, in_=knat[r0:r0 + P, :])
                        ot = outp.tile([P, D], F32, tag="ostage", name="ostage")
                        for dc in range(D // 512):
                            c_ps = psp.tile([P, 512], F32, tag="mm", name="mm", bufs=3)
                            for nb in range(NB):
                                nc.tensor.matmul(
                                    c_ps,
                                    expst[nb][:, msb * P:(msb + 1) * P],
                                    vts[nb][:, dc * 512:(dc + 1) * 512],
                                    start=(nb == 0),
                                    stop=(nb == NB - 1),
                                )
                            nc.vector.scalar_tensor_tensor(
                                out=ot[:, dc * 512:(dc + 1) * 512],
                                in0=c_ps,
                                scalar=recip_pp[:, msb:msb + 1],
                                in1=kn[:, dc * 512:(dc + 1) * 512],
                                op0=mybir.AluOpType.mult,
                                op1=mybir.AluOpType.add,
                            )
                        nc.scalar.dma_start(out=out[r0:r0 + P, :], in_=ot)

    nc.compile()
    return nc


def _get_nc():
    if "nc" not in _CACHE:
        _CACHE["nc"] = _build()
    return _CACHE["nc"]


def kernel(query_input, key_input, Wq, Wk, Wv):
    nc = _get_nc()
    bf = ml_dtypes.bfloat16
    query_input = np.asarray(query_input, dtype=np.float32)
    key_input = np.asarray(key_input, dtype=np.float32)
    Wq = np.asarray(Wq, dtype=np.float32)
    Wk = np.asarray(Wk, dtype=np.float32)
    Wv = np.asarray(Wv, dtype=np.float32)
    in_maps = []
    for b in range(B):
        in_maps.append({
            "xT": np.ascontiguousarray(query_input[b].T).astype(bf),
            "ktT": np.ascontiguousarray(key_input[b].T).astype(bf),
            "knat": np.ascontiguousarray(key_input[b]),
            "wqT": np.ascontiguousarray(Wq.T).astype(bf),
            "wkT": np.ascontiguousarray(Wk.T).astype(bf),
            "wvT": np.ascontiguousarray(Wv.T).astype(bf),
        })
    res = run_bass_kernel_spmd(nc, in_maps, list(range(B))).results
    return np.stack([res[b]["out"] for b in range(B)], axis=0)



# revision 2
# speedup vs baseline: 1.0613x; 1.0613x over previous
"""Trainium2 Bass kernel for nn_AttentionModule (dense transformer block).

Computation (per batch element b):
    q = X @ Wq.T ; k = K @ Wk.T ; v = X @ Wv.T        (X=query_input, K=key_input)
    a = softmax((k @ q.T) / sqrt(D), axis=-1)          -> (NK, NQ)
    out = a @ v + K                                    -> (NK, D)

Sharding: data-parallel over batch, one batch element per NeuronCore (B == 8).

Strategy:
  * The q/k projections are algebraically folded on the host:
        scores = K @ (Wk^T Wq) @ X^T = (K @ M) @ X^T
    so the device computes G = K@M (one 2048x1024x1024 GEMM) instead of the
    two projections q and k — 4.3 GFLOP saved per core.
  * All GEMMs run in fp8e4 with DoubleRow perf mode (two fp8 weights per PE
    cell, contraction 256 per instruction).  Operands are laid out as
    [128, 2, free] pair tiles.  M is pre-scaled by 16 and Wv by 32 on the
    host so their fp8 encodings stay in the normal range; the 1/16 folds
    into the exp() scale and the 1/32 into the softmax-denominator ones
    vector (value 32), so no extra device work.
  * Scores are built in [n, m] layout so exp(S)^T feeds the context matmul
    directly as the stationary operand; the softmax denominator is a
    ones-vector DoubleRow matmul; the normalization and the +key_input
    residual are fused into one scalar_tensor_tensor on the vector engine.
  * Everything stays SBUF-resident (no DRAM spills); fp32 is used only for
    PSUM accumulation, the residual, and the output.
"""

import numpy as np
import ml_dtypes

import concourse.tile as tile
from concourse import bacc, mybir
from concourse.bass_utils import run_bass_kernel_spmd
from concourse.masks import make_identity

B, NQ, NK, D = 8, 2048, 2048, 1024
P = 128
JD = D // (2 * P)    # 4 contraction pair-blocks (256 each)
NB = NQ // P         # 16 query-row blocks
JN = NB // 2         # 8 contraction pair-blocks over n
MC = 512             # scores chunk width (n_k columns per chunk)
NMC = NK // MC       # 4 chunks
MSB = MC // P        # 4 output row blocks per chunk

M_SCALE = 16.0       # host pre-scale of M = Wk^T Wq   (fp8 range)
V_SCALE = 32.0       # host pre-scale of Wv^T          (fp8 range)
SCALE = 1.0 / float(np.sqrt(np.float32(D)))
EXP_SCALE = SCALE / M_SCALE          # undoes M_SCALE inside exp()
ONES_VAL = V_SCALE                   # folds 1/V_SCALE into 1/colsum

F32 = mybir.dt.float32
F8 = mybir.dt.float8e4
DR = mybir.MatmulPerfMode.DoubleRow

_CACHE = {}


def _build():
    nc = bacc.Bacc("TRN2", target_bir_lowering=False, debug=False, num_devices=B)

    # pair layouts: t[128*j + p, i, c] = src[(2j+i)*128 + p, c]
    xtp = nc.dram_tensor("xtp", [JD * P, 2, NQ], F8, kind="ExternalInput").ap()
    ktp = nc.dram_tensor("ktp", [JD * P, 2, NK], F8, kind="ExternalInput").ap()
    mp = nc.dram_tensor("mp", [JD * P, 2, D], F8, kind="ExternalInput").ap()
    wvp = nc.dram_tensor("wvp", [JD * P, 2, D], F8, kind="ExternalInput").ap()
    knat = nc.dram_tensor("knat", [NK, D], F32, kind="ExternalInput").ap()
    out = nc.dram_tensor("out", [NK, D], F32, kind="ExternalOutput").ap()

    with tile.TileContext(nc) as tc:
        with (
            tc.tile_pool(name="const", bufs=1) as constp,
            tc.tile_pool(name="inp", bufs=16) as inp,
            tc.tile_pool(name="gtp", bufs=4) as gtp,
            tc.tile_pool(name="vp", bufs=8) as vp,
            tc.tile_pool(name="epp", bufs=16) as epp,
            tc.tile_pool(name="small", bufs=4) as smallp,
            tc.tile_pool(name="knp", bufs=8) as knp,
            tc.tile_pool(name="outp", bufs=4) as outp,
            tc.tile_pool(name="psum", bufs=1, space="PSUM") as psp,
        ):
            ident = constp.tile([1, 1], F32, tag="ident", name="ident")
            make_identity(nc, ident)
            # pair stride of the [:, :, 0:1] slice must be 16B-aligned
            ones = constp.tile([P, 2, 16], F8, tag="ones", name="ones")
            nc.vector.memset(ones, ONES_VAL)

            # ---------------- input loads ----------------
            # M first, then kt in column quarters (consumption order) so the
            # first G matmul group waits on ~1.5MB, not all 6MB.
            m_ = []
            for jd in range(JD):
                t = inp.tile([P, 2, D], F8, tag="m", name="m", bufs=4)
                nc.sync.dma_start(out=t, in_=mp[jd * P:(jd + 1) * P, :, :])
                m_.append(t)
            kt = [inp.tile([P, 2, NK], F8, tag="kt", name="kt", bufs=4)
                  for _ in range(JD)]
            for q in range(NMC):
                for jd in range(JD):
                    nc.sync.dma_start(
                        out=kt[jd][:, :, q * 512:(q + 1) * 512],
                        in_=ktp[jd * P:(jd + 1) * P, :, q * 512:(q + 1) * 512],
                    )
            xt = []
            for jd in range(JD):
                t = inp.tile([P, 2, NQ], F8, tag="xt", name="xt", bufs=4)
                nc.sync.dma_start(out=t, in_=xtp[jd * P:(jd + 1) * P, :, :])
                xt.append(t)
            wv = []
            for jd in range(JD):
                t = inp.tile([P, 2, D], F8, tag="wv", name="wv", bufs=4)
                nc.sync.dma_start(out=t, in_=wvp[jd * P:(jd + 1) * P, :, :])
                wv.append(t)

            gt = [gtp.tile([P, 2, NK], F8, tag="gt", name="gt") for _ in range(JD)]
            v_ = [vp.tile([P, 2, D], F8, tag="v", name="v") for _ in range(JN)]

            ci = 0

            def psum_copy(dst, src):
                # alternate PSUM->SBUF copies between the two engines that
                # have a PSUM port so neither becomes the phase bottleneck
                nonlocal ci
                ci += 1
                if ci % 2 == 0:
                    nc.vector.tensor_copy(dst, src)
                else:
                    nc.scalar.activation(
                        out=dst, in_=src, func=mybir.ActivationFunctionType.Copy
                    )

            # ---------------- phase A: G^T = M^T-pairs @ K^T ----------------
            # gt[e//2][:, e%2, m] = sum_d M[d, e*128..] * K^T[d, m]
            for e in range(D // P):
                for q in range(NMC):
                    ps = psp.tile([P, 512], F32, tag="mm", name="mm", bufs=4)
                    for jd in range(JD):
                        nc.tensor.matmul(
                            ps,
                            m_[jd][:, :, e * P:(e + 1) * P],
                            kt[jd][:, :, q * 512:(q + 1) * 512],
                            start=(jd == 0),
                            stop=(jd == JD - 1),
                            perf_mode=DR,
                        )
                    psum_copy(gt[e // 2][:, e % 2, q * 512:(q + 1) * 512], ps)

            # ---------------- phase B: V~ = X @ (32 Wv)^T ----------------
            # v_[nb//2][:, nb%2, dv] = sum_d X[nb-rows, d] * 32*Wv[dv, d]
            for nb in range(NB):
                for dc in range(D // 512):
                    ps = psp.tile([P, 512], F32, tag="mm", name="mm", bufs=4)
                    for jd in range(JD):
                        nc.tensor.matmul(
                            ps,
                            xt[jd][:, :, nb * P:(nb + 1) * P],
                            wv[jd][:, :, dc * 512:(dc + 1) * 512],
                            start=(jd == 0),
                            stop=(jd == JD - 1),
                            perf_mode=DR,
                        )
                    psum_copy(v_[nb // 2][:, nb % 2, dc * 512:(dc + 1) * 512], ps)

            # ---------------- phase C: chunked attention ----------------
            # For chunk c: S^T[n, m] -> exp -> colsum -> context + residual.
            # The softmax tail (last colsum, reciprocal, transpose) of chunk c
            # is emitted two S-groups into chunk c+1 and the context of chunk
            # c after all of S(c+1), so the PE never waits on ACT/DVE.
            state = {}

            def emit_tail(c):
                cs_ps, ept = state[c]["cs"], state[c]["ep"]
                nc.tensor.matmul(
                    cs_ps, ones[:, :, 0:1], ept[JN - 1],
                    start=False, stop=True, perf_mode=DR,
                )
                recip = smallp.tile([1, MC], F32, tag="rr", name="rr", bufs=2)
                nc.vector.reciprocal(recip, cs_ps)
                rp_ps = psp.tile([P, MSB], F32, tag="csrp", name="rp", bufs=1)
                for j in range(MSB):
                    nc.tensor.transpose(
                        rp_ps[:, j:j + 1], recip[:, j * P:(j + 1) * P], ident
                    )
                rpp = smallp.tile([P, MSB], F32, tag="rpp", name="rpp", bufs=2)
                nc.vector.tensor_copy(rpp, rp_ps)
                state[c]["rpp"] = rpp

            def emit_ctx(c):
                ept, rpp, kns = state[c]["ep"], state[c]["rpp"], state[c]["kn"]
                m0 = c * MC
                for msb in range(MSB):
                    ot = outp.tile([P, D], F32, tag="ot", name="ot", bufs=4)
                    for dc in range(D // 512):
                        ps = psp.tile([P, 512], F32, tag="ctx", name="ctx", bufs=3)
                        for jn in range(JN):
                            nc.tensor.matmul(
                                ps,
                                ept[jn][:, :, msb * P:(msb + 1) * P],
                                v_[jn][:, :, dc * 512:(dc + 1) * 512],
                                start=(jn == 0),
                                stop=(jn == JN - 1),
                                perf_mode=DR,
                            )
                        nc.vector.scalar_tensor_tensor(
                            out=ot[:, dc * 512:(dc + 1) * 512],
                            in0=ps,
                            scalar=rpp[:, msb:msb + 1],
                            in1=kns[msb][:, dc * 512:(dc + 1) * 512],
                            op0=mybir.AluOpType.mult,
                            op1=mybir.AluOpType.add,
                        )
                    nc.scalar.dma_start(
                        out=out[m0 + msb * P:m0 + (msb + 1) * P, :], in_=ot
                    )
                del state[c]

            for c in range(NMC):
                m0 = c * MC
                kns = []
                for msb in range(MSB):
                    t = knp.tile([P, D], F32, tag="kn", name="kn", bufs=8)
                    nc.sync.dma_start(
                        out=t, in_=knat[m0 + msb * P:m0 + (msb + 1) * P, :]
                    )
                    kns.append(t)
                ept = [epp.tile([P, 2, MC], F8, tag="ep", name="ep", bufs=16)
                       for _ in range(JN)]
                cs_ps = psp.tile([1, MC], F32, tag="csrp", name="cs", bufs=1)
                state[c] = {"cs": cs_ps, "ep": ept, "kn": kns}

                for nb in range(NB):
                    ps = psp.tile([P, MC], F32, tag="mm", name="mm", bufs=4)
                    for jd in range(JD):
                        nc.tensor.matmul(
                            ps,
                            xt[jd][:, :, nb * P:(nb + 1) * P],
                            gt[jd][:, :, m0:m0 + MC],
                            start=(jd == 0),
                            stop=(jd == JD - 1),
                            perf_mode=DR,
                        )
                    nc.scalar.activation(
                        out=ept[nb // 2][:, nb % 2, :], in_=ps,
                        func=mybir.ActivationFunctionType.Exp, scale=EXP_SCALE,
                    )
                    if nb == 1 and c > 0:
                        emit_tail(c - 1)
                    # colsum for pair jn lags its exps by two S-groups so the
                    # exp -> colsum semaphore never gates the PE
                    if nb >= 3 and nb % 2 == 1:
                        jn = (nb - 3) // 2
                        nc.tensor.matmul(
                            cs_ps, ones[:, :, 0:1], ept[jn],
                            start=(jn == 0), stop=False, perf_mode=DR,
                        )
                if c > 0:
                    emit_ctx(c - 1)
            emit_tail(NMC - 1)
            emit_ctx(NMC - 1)

    nc.compile()
    return nc


def _get_nc():
    if "nc" not in _CACHE:
        _CACHE["nc"] = _build()
    return _CACHE["nc"]


def _pair(a):
    """[D, C] -> [JD*P, 2, C] with t[128j+p, i, c] = a[(2j+i)*128+p, c]."""
    Dd, C = a.shape
    return np.ascontiguousarray(
        a.reshape(JD, 2, P, C).transpose(0, 2, 1, 3).reshape(JD * P, 2, C)
    )


def make_in_maps(query_input, key_input, Wq, Wk, Wv):
    f8 = ml_dtypes.float8_e4m3
    query_input = np.asarray(query_input, dtype=np.float32)
    key_input = np.asarray(key_input, dtype=np.float32)
    Wq = np.asarray(Wq, dtype=np.float32)
    Wk = np.asarray(Wk, dtype=np.float32)
    Wv = np.asarray(Wv, dtype=np.float32)

    m_pre = _pair(M_SCALE * (Wk.T @ Wq)).astype(f8)
    wv_pre = _pair(V_SCALE * Wv.T).astype(f8)
    in_maps = []
    for b in range(B):
        in_maps.append({
            "xtp": _pair(query_input[b].T.copy()).astype(f8),
            "ktp": _pair(key_input[b].T.copy()).astype(f8),
            "mp": m_pre,
            "wvp": wv_pre,
            "knat": np.ascontiguousarray(key_input[b]),
        })
    return in_maps


def kernel(query_input, key_input, Wq, Wk, Wv):
    nc = _get_nc()
    in_maps = make_in_maps(query_input, key_input, Wq, Wk, Wv)
    res = run_bass_kernel_spmd(nc, in_maps, list(range(B))).results
    return np.stack([res[b]["out"] for b in range(B)], axis=0)
